# revision 45
# baseline (speedup 1.0000x reference)
"""Multi-head self-attention (B=2, N=2048, C=1024, H=16, D=64) on 8 trn2 cores.

Sharding: core c handles batch b = c//4 and the 4 heads [4*(c%4), 4*(c%4)+4).
Host pre-transposes x and the weight slices; per-core partial outputs are
summed on the host and the output bias is added there.

Device kernel (per core): attention runs in bf16 (PE streams bf16 at one
row/cycle, same as fp32r, and it halves HBM/SBUF traffic); PSUM accumulation
stays fp32.
  phase 1: qkT/V projections per 512-column chunk of x; the S matmuls and
           exps of the first two attention blocks (query tile 0, both head
           pairs) are hoisted in chunk by chunk so ScalarE works during the
           projections; their exp outputs stay resident (pth) until phase 2.
  phase 2: one flat software pipeline over (query tile, head pair, j block):
           PE issues S two steps ahead of PV so ScalarE's exp stream never
           starves; the hoisted blocks' PV debt drains at up to 4 PVs/step;
           out-projection (head pairs packed so K=128) and the rank-1
           1/denominator broadcasts pop into PE slack slots as fillers, with
           a few ready units held back to cover the tail.
"""

import os
from collections import deque

import ml_dtypes
import numpy as np

import concourse.mybir as mybir
import concourse.tile as tile
from concourse import bacc
from concourse.bass_utils import run_bass_kernel_spmd

F32 = mybir.dt.float32
F32R = mybir.dt.float32r
BF16 = mybir.dt.bfloat16

B, N, C = 2, 2048, 1024
H, D = 16, 64
HPC = 4            # heads per core
P = 128
FD = 512           # matmul free-dim tile
KB = C // P        # 8 contraction blocks for the projections
NT = N // FD       # 4 free tiles over the sequence
NJB = N // P       # 16 j blocks in attention
HB = 2             # blocks (query tile 0, both prs) hoisted into phase 1

# schedule tuning knobs (env-overridable for experiments)
BGT_HI = int(os.environ.get("BGT_HI", "8"))    # PV catch-up: lag for budget 4
BGT_LO = int(os.environ.get("BGT_LO", "4"))    # PV catch-up: lag for budget 2
REP_SLOTS = tuple(int(v) for v in
                  os.environ.get("REP_SLOTS", "7,8").split(","))
FQ_LO = int(os.environ.get("FQ_LO", "9"))      # out-proj filler drain slots
FQ_HI = int(os.environ.get("FQ_HI", "12"))


def build_nc(repeat: int = 1) -> bacc.Bacc:
    nc = bacc.Bacc("TRN2", target_bir_lowering=False, debug=False)

    xT = nc.dram_tensor("xT", [C, N], BF16, kind="ExternalInput").ap()
    wqkvT = nc.dram_tensor("wqkvT", [C, 3 * HPC * D], BF16,
                           kind="ExternalInput").ap()
    woutT = nc.dram_tensor("woutT", [P, 2, C], BF16, kind="ExternalInput").ap()
    ones2 = nc.dram_tensor("ones2", [2, P], F32, kind="ExternalInput").ap()
    y = nc.dram_tensor("y", [N, C], BF16, kind="ExternalOutput").ap()

    xT_r = xT.rearrange("(o p) n -> p o n", p=P)          # [128, 8, 2048]
    wqkvT_r = wqkvT.rearrange("(o p) f -> p o f", p=P)    # [128, 8, 768]

    with tile.TileContext(nc) as tc:
        with (
            tc.tile_pool(name="w_pool", bufs=1) as w_pool,
            tc.tile_pool(name="qk_pool", bufs=1) as qk_pool,
            tc.tile_pool(name="v_pool", bufs=1) as v_pool,
            tc.tile_pool(name="o_pool", bufs=1) as o_pool,
            tc.tile_pool(name="x_pool", bufs=2) as x_pool,
            tc.tile_pool(name="pt_pool", bufs=6) as pt_pool,
            tc.tile_pool(name="pth_pool", bufs=1) as pth_pool,
            tc.tile_pool(name="y_pool", bufs=4) as y_pool,
            tc.tile_pool(name="nrm_pool", bufs=2) as nrm_pool,
            tc.tile_pool(name="ps", bufs=1, space="PSUM") as ps,
        ):
            # DMA priority order (transfers serialize through the DMA
            # engines): first x chunk halves interleaved with the wqkv weight
            # halves they unblock, then the late-needed tensors.
            wq_p = [w_pool.tile([P, 2, 3 * HPC * D], BF16, name=f"wqp{i}")
                    for i in range(4)]
            x0_p = [x_pool.tile([P, 2, FD], BF16, tag=f"x0p{i}", bufs=1,
                                name=f"x0p{i}")
                    for i in range(4)]
            for i in range(4):
                nc.sync.dma_start(wq_p[i], wqkvT_r[:, 2 * i:2 * i + 2, :])
                nc.scalar.dma_start(x0_p[i], xT_r[:, 2 * i:2 * i + 2, 0:FD])
            wo_sb = w_pool.tile([P, 2, C], BF16)
            nc.scalar.dma_start(wo_sb, woutT)
            ones2_sb = w_pool.tile([2, P], F32R)
            nc.sync.dma_start(ones2_sb, ones2.bitcast(F32R))

            qkT_sb = qk_pool.tile([P, 4, N], BF16)       # q01 | q23 | k01 | k23
            V_sb = v_pool.tile([P, NJB, HPC, D + 1], BF16)
            oT_sb = o_pool.tile([P, 2, N], BF16)         # [hh*64+d, pr, n]
            # resident exp outputs for the hoisted blocks
            pth = pth_pool.tile([P, HB * NJB, 2 * FD], BF16)
            nc.vector.memset(V_sb[:, :, :, D:D + 1], 1.0)

            blocks = [(itl, pr) for itl in range(NT) for pr in range(2)]
            TOT = len(blocks) * NJB
            HS = HB * NJB          # hoisted steps

            for _rep in range(repeat):
                fq = deque()     # out-proj filler units for PE slack slots
                repq = deque()   # 1/denominator broadcast+mul units
                tailq = []       # ready units held back to cover the tail
                ot_blk, sts, pts = {}, {}, {}

                def drain(nmax):
                    k = 0
                    while fq and k < nmax:
                        fq.popleft()()
                        k += 1

                def S_step(s):
                    (itl, pr), jb = blocks[s // NJB], s % NJB
                    st = ps.tile([P, 2 * FD], F32, tag="st", bufs=2, name="st")
                    for hh in range(2):
                        lo = hh * D
                        nc.tensor.matmul(
                            st[:, hh * FD:(hh + 1) * FD],
                            lhsT=qkT_sb[lo:lo + D, 2 + pr, jb * P:(jb + 1) * P],
                            rhs=qkT_sb[lo:lo + D, pr, itl * FD:(itl + 1) * FD],
                            start=True, stop=True)
                    sts[s] = st

                def E_step(s):
                    if s < HS:
                        pt = pth[:, s, :]
                    else:
                        pt = pt_pool.tile([P, 2 * FD], BF16, tag="pt",
                                          name="pt")
                    nc.scalar.activation(
                        pt, sts.pop(s), mybir.ActivationFunctionType.Exp,
                        scale=0.125)
                    pts[s] = pt

                def push_norm(ot_h, pr, itl, last=False):
                    """ot's PSUM readers (rin + oT evictions) come first in
                    the DVE queue so the next block's first PV (WAR on the ot
                    slot) unblocks quickly. For the final block the scalar
                    chain runs on ScalarE (idle by then), in parallel with
                    DVE's oT evictions."""
                    osl = oT_sb[:, pr, itl * FD:(itl + 1) * FD]
                    rec2s = []
                    for hh in range(2):
                        rin = nrm_pool.tile([1, FD], F32, tag="rin",
                                            name="rin")
                        if last:
                            nc.scalar.activation(
                                rin, ot_h[hh][D:D + 1, :],
                                mybir.ActivationFunctionType.Copy)
                            nc.scalar.activation(
                                osl[hh * D:(hh + 1) * D], ot_h[hh][0:D, :],
                                mybir.ActivationFunctionType.Copy)
                        else:
                            nc.vector.tensor_copy(rin, ot_h[hh][D:D + 1, :])
                            nc.vector.tensor_copy(
                                osl[hh * D:(hh + 1) * D], ot_h[hh][0:D, :])
                        rec = nrm_pool.tile([1, FD], F32, tag="rec",
                                            name="rec")
                        nc.vector.reciprocal_approx_fast(out=rec, in_=rin)
                        rec2 = nrm_pool.tile([1, FD], F32R, tag="rec2",
                                             name="rec2")
                        nc.vector.tensor_copy(rec2, rec)
                        rec2s.append(rec2)

                    def rep_mul(hh, rec2):
                        rep = ps.tile([D, FD], F32, tag="mm", bufs=2,
                                      name="rep")
                        nc.tensor.matmul(rep, lhsT=ones2_sb[0:1, 0:D],
                                         rhs=rec2, start=True, stop=True)
                        nc.vector.tensor_mul(out=osl[hh * D:(hh + 1) * D],
                                             in0=osl[hh * D:(hh + 1) * D],
                                             in1=rep)
                    for hh in range(2):
                        repq.append(lambda hh=hh, rec2=rec2s[hh]:
                                    rep_mul(hh, rec2))

                def push_outproj(itl):
                    """y rows of this query tile; heads packed so K=128. The
                    last tile's PSUM evictions go through ScalarE (idle by
                    then) so DVE isn't the tail's critical path."""
                    for i4 in range(4):
                        it = itl * 4 + i4
                        y_t = y_pool.tile([P, C], BF16, tag="yt")
                        for o2 in range(2):
                            def unit(y_t=y_t, it=it, o2=o2, itl=itl):
                                py = ps.tile([P, FD], F32, tag="mm", bufs=2,
                                             name="py")
                                for g in range(2):
                                    nc.tensor.matmul(
                                        py,
                                        lhsT=oT_sb[:, g, it * P:(it + 1) * P],
                                        rhs=wo_sb[:, g,
                                                  o2 * FD:(o2 + 1) * FD],
                                        start=(g == 0), stop=(g == 1))
                                ysl = y_t[:, o2 * FD:(o2 + 1) * FD]
                                tail_unit = (itl == NT - 1
                                             or (itl == NT - 2
                                                 and it - itl * 4 >= 2))
                                if tail_unit and (it + o2) % 2 == 0:
                                    nc.scalar.activation(
                                        ysl, py,
                                        mybir.ActivationFunctionType.Copy)
                                else:
                                    nc.vector.tensor_copy(ysl, py)
                                eng = nc.sync if (it + o2) % 2 == 0 \
                                    else nc.scalar
                                eng.dma_start(
                                    y[it * P:(it + 1) * P,
                                      o2 * FD:(o2 + 1) * FD], ysl)
                            if itl == NT - 2 and i4 >= 2:
                                tailq.append(unit)
                            else:
                                fq.append(unit)

                def PV_step(s):
                    (itl, pr), jb = blocks[s // NJB], s % NJB
                    if jb == 0:
                        ot_blk[s // NJB] = [
                            ps.tile([D + 1, FD], F32, tag="ot", bufs=2,
                                    name=f"ot{pr}{itl}{hh}") for hh in range(2)]
                    ot_h = ot_blk[s // NJB]
                    pt = pts.pop(s)
                    for hh in range(2):
                        nc.tensor.matmul(
                            ot_h[hh],
                            lhsT=V_sb[:, jb, 2 * pr + hh, :],
                            rhs=pt[:, hh * FD:(hh + 1) * FD],
                            start=(jb == 0), stop=(jb == NJB - 1))
                    if jb == NJB - 1:
                        push_norm(ot_h, pr, itl, last=(s == TOT - 1))
                        del ot_blk[s // NJB]
                        if pr == 1:
                            push_outproj(itl)

                # ---------------- phase 1: projections ----------------
                def proj_qk(nt, mt, xof):
                    pq = ps.tile([P, FD], F32, tag="mm", bufs=2, name="pq")
                    for kb in range(KB):
                        nc.tensor.matmul(
                            pq, lhsT=wq_p[kb // 2][:, kb % 2,
                                              mt * P:(mt + 1) * P],
                            rhs=xof(kb),
                            start=(kb == 0), stop=(kb == KB - 1))
                    nc.vector.tensor_copy(
                        qkT_sb[:, mt, nt * FD:(nt + 1) * FD], pq)

                def hoist_se(pr, nt):
                    for jb in range(4 * nt, 4 * nt + 4):
                        S_step(pr * NJB + jb)
                        E_step(pr * NJB + jb)

                for nt in range(NT):
                    if nt == 0:
                        xof = lambda kb: x0_p[kb // 2][:, kb % 2, :]
                    else:
                        xc = x_pool.tile([P, KB, FD], BF16, tag="xc")
                        eng = nc.sync if nt % 2 == 0 else nc.scalar
                        eng.dma_start(xc, xT_r[:, :, nt * FD:(nt + 1) * FD])
                        xof = lambda kb, xc=xc: xc[:, kb, :]
                    mts = [2, 0, 1, 3] if nt == 0 else [2, 3, 0, 1]
                    for mi, mt in enumerate(mts):
                        proj_qk(nt, mt, xof)
                        if nt == 0 and mi == 1:
                            hoist_se(0, nt)
                        elif nt == 0 and mi == 3:
                            hoist_se(1, nt)
                        elif nt > 0 and mt == 2:
                            hoist_se(0, nt)
                        elif nt > 0 and mt == 3:
                            hoist_se(1, nt)
                    for i4 in range(4):
                        it = nt * 4 + i4
                        pv = ps.tile([P, HPC * D], F32, tag="mm", bufs=2,
                                     name="pv")
                        for kb in range(KB):
                            nc.tensor.matmul(
                                pv,
                                lhsT=xof(kb)[:, i4 * P:(i4 + 1) * P],
                                rhs=wq_p[kb // 2][:, kb % 2,
                                                  2 * HPC * D:3 * HPC * D],
                                start=(kb == 0), stop=(kb == KB - 1))
                        nc.vector.tensor_copy(
                            V_sb[:, it, :, 0:D],
                            pv.rearrange("p (h d) -> p h d", d=D))

                # -------- phase 2: flat software-pipelined attention --------
                pv_ptr = 0
                for s in range(HS, TOT + 2):
                    if s < TOT:
                        S_step(s)
                    lag = s - 2 - pv_ptr
                    budget = (4 if lag > BGT_HI else
                              (2 if lag > BGT_LO else 1))
                    while pv_ptr <= min(s - 2, TOT - 1) and budget > 0:
                        PV_step(pv_ptr)
                        pv_ptr += 1
                        budget -= 1
                        # fixed drain slots per PV block: rep units pop a few
                        # steps after their push (their DVE reciprocal chain
                        # needs ~3 steps), out-proj units after that;
                        # block-boundary steps stay clear
                        m = pv_ptr % 16
                        if m in REP_SLOTS and repq:
                            repq.popleft()()
                        elif FQ_LO <= m <= FQ_HI and fq:
                            fq.popleft()()
                    if HS <= s - 1 < TOT:
                        E_step(s - 1)
                while pv_ptr < TOT:
                    PV_step(pv_ptr)
                    pv_ptr += 1
                for unit in tailq:
                    unit()
                while repq:
                    repq.popleft()()
                drain(len(fq))

    nc.finalize()
    return nc


def shard_inputs(x, w_qkv, w_out):
    """Full inputs -> list of 8 per-core input maps (host-side prep)."""
    bf = ml_dtypes.bfloat16
    x = np.asarray(x, dtype=np.float32)
    w_qkv = np.asarray(w_qkv, dtype=np.float32)
    w_out = np.asarray(w_out, dtype=np.float32)
    ones2 = np.zeros((2, P), np.float32)
    ones2[0, 0:D] = 1.0
    ones2[1, D:2 * D] = 1.0
    in_maps = []
    for c in range(8):
        b, hp = c // 4, c % 4
        rows = np.concatenate(
            [w_qkv[q * C + hp * HPC * D:(q * C + (hp + 1) * HPC * D)]
             for q in range(3)], axis=0)                      # [768, C]
        # out-proj weights packed in head pairs: [hh*64+d, pr, C]
        wo = w_out[:, hp * HPC * D:(hp + 1) * HPC * D].T      # [256, C]
        wo = wo.reshape(2, 2, D, C).transpose(1, 2, 0, 3)     # [hh, 64, pr, C]
        in_maps.append({
            "ones2": ones2,
            "xT": np.ascontiguousarray(x[b].T).astype(bf),     # [C, N]
            "wqkvT": np.ascontiguousarray(rows.T).astype(bf),  # [C, 768]
            "woutT": np.ascontiguousarray(wo.reshape(P, 2, C)).astype(bf),
        })
    return in_maps


def combine_outputs(ys, b_out):
    b_out = np.asarray(b_out, dtype=np.float32)
    ys = [np.asarray(t, dtype=np.float32) for t in ys]
    out0 = ys[0] + ys[1] + ys[2] + ys[3]
    out1 = ys[4] + ys[5] + ys[6] + ys[7]
    return np.stack([out0, out1], axis=0) + b_out[None, None, :]


_NC = None


def kernel(x, w_qkv, w_out, b_out):
    global _NC
    if _NC is None:
        _NC = build_nc()
    in_maps = shard_inputs(x, w_qkv, w_out)
    res = run_bass_kernel_spmd(_NC, in_maps, core_ids=list(range(8)))
    ys = [res.results[c]["y"] for c in range(8)]
    return combine_outputs(ys, b_out).astype(np.float32)

